# revision 1
# baseline (speedup 1.0000x reference)
"""Multi-head attention (b=4, n=2048, dim=512, h=8, dh=64) on 8 trn2 cores.

Sharding: core c handles batch b=c//2 and query rows
[half*1024, (half+1)*1024) with half=c%2. K/V (from x_prev) are computed
redundantly on both of a batch's cores (cheap vs. attention). No collectives.

Per-core kernel (bf16 operands, fp32 PSUM accumulation):
  QT[inner, nq]  = w_q-tiles  (lhsT) @ x^T          (q in transposed layout)
  KT[inner, nk]  = w_kv-tiles (lhsT) @ x_prev^T
  V [nk, inner]  = x_prev^T-tiles (lhsT) @ w_kv[:, v]  (natural layout,
                                                        + ones column/head)
  ST[j, i]       = KT_h-tile (lhsT, K=dh=64) @ QT_h  (scores transposed;
                   two heads row-tiled in the PE at partitions 0/64)
  PT             = exp(ST * scale)    (no max subtraction: |s*scale| < ~8)
  PV[dh+1, i]    = sum_j V_h|ones (lhsT) @ PT_h      (row dh = sum_j P = l)
  l -> (dma to partitions 0/1) -> r = 1/l -> rb_h = ones-matmul broadcast
  AOT[0:64, h]   = PV[0:dh] * rb_h                   (per-head, partitions 0-63)
  out[i, d]      = sum_h AOT_h-tiles (lhsT, K=64) @ w_out_h + ones @ b_out
"""

import numpy as np
import ml_dtypes

B, N, DIM = 4, 2048, 512
H, DH, INNER = 8, 64, 512
NCORES = 8

_BUILT = None


def build_module(dim=DIM, h=H, nq=N // 2, nk=N, compile_module=True, reps=1,
                 stub=frozenset()):
    """Build the per-core attention module. dim % 128 == 0, h % 2 == 0,
    nq % 512 == 0, nk % 128 == 0. reps>1 repeats the whole compute body
    (timing calibration only)."""
    import concourse.mybir as mybir
    import concourse.tile as tile
    from concourse import bacc

    CDT = mybir.dt.bfloat16
    FDT = mybir.dt.float32
    Exp = mybir.ActivationFunctionType.Exp

    inner = h * DH
    nkt = dim // 128          # contraction tiles for projections
    npr = h // 2              # head pairs (= inner // 128 slices of KT/QT)
    nj = nk // 128            # key tiles
    nqc = nq // 512           # query chunks
    VW = DH + 1               # 65: per-head v columns + ones column
    scale = DH ** -0.5

    nc = bacc.Bacc("TRN2", target_bir_lowering=False, debug=False,
                   num_devices=NCORES)

    xt_d = nc.declare_dram_parameter("xt", [dim, nq], CDT, isOutput=False)
    xpt_d = nc.declare_dram_parameter("xpt", [dim, nk], CDT, isOutput=False)
    wq_d = nc.declare_dram_parameter("wq", [dim, inner], CDT, isOutput=False)
    wkv_d = nc.declare_dram_parameter("wkv", [dim, 2 * inner], CDT,
                                      isOutput=False)
    # w_out pre-arranged on host: wout2[d, h, :] = w_out[h*64+d, :]
    wout_d = nc.declare_dram_parameter("wout", [DH, h, dim], CDT,
                                       isOutput=False)
    bout_d = nc.declare_dram_parameter("bout", [1, dim], CDT, isOutput=False)
    out_d = nc.declare_dram_parameter("out", [nq, dim], FDT, isOutput=True)
    # DRAM bounce rows for the 1/l partition-broadcast (SBUF APs cannot
    # have a zero-step partition dim; DRAM APs can).
    rsc_d = nc.dram_tensor("rscratch", [h * nq // 512, 512], FDT)

    import contextlib
    with tile.TileContext(nc) as tc, contextlib.ExitStack() as stack:
        consts = stack.enter_context(tc.tile_pool(name="consts", bufs=1))
        acts = stack.enter_context(tc.tile_pool(name="acts", bufs=1))

        # ---- constants / weights ----------------------------------------
        wq_sb = consts.tile([128, nkt, inner], CDT)
        wkv_sb = consts.tile([128, nkt, 2 * inner], CDT)
        wout_sb = consts.tile([DH, h, dim], CDT)
        bout_sb = consts.tile([1, dim], CDT)
        ones_sb = consts.tile([1, 128], CDT)

        for k in range(nkt):
            nc.sync.dma_start(
                out=wq_sb[:, k, :],
                in_=wq_d.ap().rearrange("(t p) o -> p t o", p=128)[:, k, :])
            nc.sync.dma_start(
                out=wkv_sb[:, k, :],
                in_=wkv_d.ap().rearrange("(t p) o -> p t o", p=128)[:, k, :])
        nc.sync.dma_start(out=wout_sb[:, :, :], in_=wout_d.ap())
        nc.sync.dma_start(out=bout_sb[:, :], in_=bout_d.ap())
        nc.vector.memset(ones_sb[:, :], 1.0)

        # ---- activations -------------------------------------------------
        xt_sb = acts.tile([128, nkt, nq], CDT)
        xpt_sb = acts.tile([128, nkt, nk], CDT)
        for k in range(nkt):
            nc.sync.dma_start(
                out=xt_sb[:, k, :],
                in_=xt_d.ap().rearrange("(t p) n -> p t n", p=128)[:, k, :])
            nc.sync.dma_start(
                out=xpt_sb[:, k, :],
                in_=xpt_d.ap().rearrange("(t p) n -> p t n", p=128)[:, k, :])

        qt_sb = acts.tile([128, npr, nq], CDT)    # [inner-slice, nq]
        kt_sb = acts.tile([128, npr, nk], CDT)    # [inner-slice, nk]
        v_sb = acts.tile([128, nj, h * VW], CDT)  # [key-tile, h*(dh+1)]
        aot_sb = acts.tile([DH, h, nq], CDT)      # [dh, head, nq]

        for hh in range(h):  # ones columns of V
            nc.vector.memset(v_sb[:, :, hh * VW + DH:hh * VW + DH + 1], 1.0)

        for _rep in range(reps):
            # KT/QT slice 0 first so attention can start early; V interleaved.
            kw = min(512, nk)   # kt projection chunk width
            order = []
            for s in range(npr):
                for c in range(nk // kw):
                    order.append(("kt", s, c))
                for c in range(nqc):
                    order.append(("qt", s, c))
                if s == 0:
                    for j in range(nj):
                        order.append(("v", j, 0))
            proj_scope = tc.tile_pool(name="proj_ps", bufs=4, space="PSUM")
            proj_ps = proj_scope.__enter__()
            for kind, a, c in order:
                ps = proj_ps.tile([128, 512], FDT, tag="mm")
                if kind == "kt":
                    for k in range(nkt):
                        nc.tensor.matmul(
                            ps[:, 0:kw], lhsT=wkv_sb[:, k, a * 128:(a + 1) * 128],
                            rhs=xpt_sb[:, k, c * kw:(c + 1) * kw],
                            start=(k == 0), stop=(k == nkt - 1))
                    nc.vector.tensor_copy(
                        out=kt_sb[:, a, c * kw:(c + 1) * kw], in_=ps[:, 0:kw])
                elif kind == "qt":
                    for k in range(nkt):
                        nc.tensor.matmul(
                            ps[:, :], lhsT=wq_sb[:, k, a * 128:(a + 1) * 128],
                            rhs=xt_sb[:, k, c * 512:(c + 1) * 512],
                            start=(k == 0), stop=(k == nkt - 1))
                    nc.vector.tensor_copy(
                        out=qt_sb[:, a, c * 512:(c + 1) * 512], in_=ps[:, :])
                else:  # v: natural layout, lhsT = xpt token-tile
                    for k in range(nkt):
                        nc.tensor.matmul(
                            ps[:, 0:inner],
                            lhsT=xpt_sb[:, k, a * 128:(a + 1) * 128],
                            rhs=wkv_sb[:, k, inner:2 * inner],
                            start=(k == 0), stop=(k == nkt - 1))
                    nc.vector.tensor_copy(
                        out=v_sb[:, a, :].rearrange(
                            "p (g x) -> p g x", x=VW)[:, :, 0:DH],
                        in_=ps[:, 0:inner].rearrange("p (g x) -> p g x", x=DH))

            proj_scope.__exit__(None, None, None)

            # ---- attention ---------------------------------------------------
            attn_stack = contextlib.ExitStack()
            st_ps = attn_stack.enter_context(
                tc.tile_pool(name="st_ps", bufs=2, space="PSUM"))
            acc_ps = attn_stack.enter_context(
                tc.tile_pool(name="acc_ps", bufs=4, space="PSUM"))
            pt_pool = attn_stack.enter_context(tc.tile_pool(name="pt", bufs=2))
            lr_pool = attn_stack.enter_context(tc.tile_pool(name="lr", bufs=3))

            for c in range(nqc):          # query chunk of 512
                for p in range(npr):      # head pair (2p, 2p+1)
                    h0, h1 = 2 * p, 2 * p + 1
                    pt = pt_pool.tile([128, nj, 1024], CDT, tag="pt")
                    for j in range(nj):
                        st = st_ps.tile([128, 1024], FDT, tag="st")
                        nc.tensor.matmul(
                            st[:, 0:512],
                            lhsT=kt_sb[0:64, p, j * 128:(j + 1) * 128],
                            rhs=qt_sb[0:64, p, c * 512:(c + 1) * 512],
                            start=True, stop=True)
                        nc.tensor.matmul(
                            st[:, 512:1024],
                            lhsT=kt_sb[64:128, p, j * 128:(j + 1) * 128],
                            rhs=qt_sb[64:128, p, c * 512:(c + 1) * 512],
                            start=True, stop=True)
                        if "noexp" in stub:
                            nc.vector.tensor_copy(out=pt[:, j, :],
                                                  in_=st[:, :])
                        else:
                            nc.scalar.activation(out=pt[:, j, :], in_=st[:, :],
                                                 func=Exp, scale=scale)

                    pv0 = acc_ps.tile([128, 512], FDT, tag="acc")
                    pv1 = acc_ps.tile([128, 512], FDT, tag="acc")
                    for j in range(nj):
                        nc.tensor.matmul(
                            pv0[0:VW, :], lhsT=v_sb[:, j, h0 * VW:(h0 + 1) * VW],
                            rhs=pt[:, j, 0:512],
                            start=(j == 0), stop=(j == nj - 1))
                        nc.tensor.matmul(
                            pv1[0:VW, :], lhsT=v_sb[:, j, h1 * VW:(h1 + 1) * VW],
                            rhs=pt[:, j, 512:1024],
                            start=(j == 0), stop=(j == nj - 1))

                    if "nonorm" in stub:
                        nc.vector.tensor_copy(
                            out=aot_sb[:, h0, c * 512:(c + 1) * 512],
                            in_=pv0[0:DH, :])
                        nc.vector.tensor_copy(
                            out=aot_sb[:, h1, c * 512:(c + 1) * 512],
                            in_=pv1[0:DH, :])
                        continue
                    # softmax denominators: 1/l on lane DH, then a step-0
                    # partition DMA broadcasts it to 64 partitions in SBUF.
                    ra_sb = lr_pool.tile([DH + 1, 512], FDT, tag="ra")
                    rc_sb = lr_pool.tile([DH + 1, 512], FDT, tag="rc")
                    nc.vector.reciprocal(out=ra_sb[DH:DH + 1, :],
                                         in_=pv0[DH:DH + 1, :])
                    nc.vector.reciprocal(out=rc_sb[DH:DH + 1, :],
                                         in_=pv1[DH:DH + 1, :])
                    idx = (c * npr + p) * 2
                    nc.sync.dma_start(out=rsc_d.ap()[idx:idx + 1, :],
                                      in_=ra_sb[DH:DH + 1, :])
                    nc.sync.dma_start(out=rsc_d.ap()[idx + 1:idx + 2, :],
                                      in_=rc_sb[DH:DH + 1, :])
                    rb0 = lr_pool.tile([DH, 512], FDT, tag="rb0")
                    rb1 = lr_pool.tile([DH, 512], FDT, tag="rb1")
                    nc.gpsimd.dma_start(
                        out=rb0[:, :],
                        in_=rsc_d.ap()[idx:idx + 1, :].to_broadcast([DH, 512]))
                    nc.gpsimd.dma_start(
                        out=rb1[:, :],
                        in_=rsc_d.ap()[idx + 1:idx + 2, :].to_broadcast([DH, 512]))
                    nc.vector.tensor_mul(
                        aot_sb[:, h0, c * 512:(c + 1) * 512],
                        pv0[0:DH, :], rb0[:, :])
                    nc.vector.tensor_mul(
                        aot_sb[:, h1, c * 512:(c + 1) * 512],
                        pv1[0:DH, :], rb1[:, :])

                # ---- output projection for this chunk (4 row-tiles of 128) ---
                for t in range(4 * c, 4 * c + 4):
                    f = acc_ps.tile([128, 512], FDT, tag="acc")
                    for hh in range(h):
                        nc.tensor.matmul(
                            f[:, 0:dim],
                            lhsT=aot_sb[:, hh, t * 128:(t + 1) * 128],
                            rhs=wout_sb[:, hh, :],
                            start=(hh == 0), stop=False)
                    nc.tensor.matmul(f[:, 0:dim], lhsT=ones_sb[:, :],
                                     rhs=bout_sb[:, :], start=False, stop=True)
                    fo = lr_pool.tile([128, dim], FDT, tag="fo")
                    nc.vector.tensor_copy(out=fo[:, :], in_=f[:, 0:dim])
                    nc.sync.dma_start(
                        out=out_d.ap()[t * 128:(t + 1) * 128, :], in_=fo[:, :])
            attn_stack.close()

    if compile_module:
        nc.compile()
    return nc


def host_inputs(x, x_prev, w_q, w_kv, w_out, b_out, ncores=NCORES):
    """Shard + lay out the full inputs into per-core input maps."""
    bf16 = ml_dtypes.bfloat16
    b, n, dim = x.shape
    inner = w_q.shape[1]
    h = inner // DH
    nq = (b * n) // ncores
    halves = ncores // b
    wq = np.ascontiguousarray(w_q).astype(bf16)
    wkv = np.ascontiguousarray(w_kv).astype(bf16)
    wout = np.ascontiguousarray(
        w_out.reshape(h, DH, dim).transpose(1, 0, 2)).astype(bf16)
    bout = np.ascontiguousarray(b_out).reshape(1, dim).astype(bf16)
    in_maps = []
    for c in range(ncores):
        bb, half = c // halves, c % halves
        xt = np.ascontiguousarray(
            x[bb, half * nq:(half + 1) * nq, :].T).astype(bf16)
        xpt = np.ascontiguousarray(x_prev[bb].T).astype(bf16)
        in_maps.append(dict(xt=xt, xpt=xpt, wq=wq, wkv=wkv, wout=wout,
                            bout=bout))
    return in_maps


def _get_module():
    global _BUILT
    if _BUILT is None:
        _BUILT = build_module()
    return _BUILT


def kernel(x, x_prev, w_q, w_kv, w_out, b_out):
    from concourse.bass_utils import run_bass_kernel_spmd

    nc = _get_module()
    in_maps = host_inputs(x, x_prev, w_q, w_kv, w_out, b_out)
    res = run_bass_kernel_spmd(nc, in_maps, core_ids=list(range(NCORES)))

    nq = N // 2
    out = np.empty((B, N, DIM), np.float32)
    for c in range(NCORES):
        b, half = c // 2, c % 2
        out[b, half * nq:(half + 1) * nq, :] = res.results[c]["out"]
    return out

